# revision 13
# baseline (speedup 1.0000x reference)
"""Trainium2 Bass kernel for a dense multi-head attention layer.

Problem (hardcoded shapes):
    hidden_states [2, 2048, 2048] fp32, attention_mask [2,1,1,2048] int32 (all ones),
    Wq/Wk/Wv/Wo [2048, 2048] fp32, biases [2048] fp32 (zeros in practice).
    out = MHA(hidden) with H=16 heads, head_dim=128.

Sharding: 8 cores = 2 batches x 4 head-groups (4 heads per core, tensor
parallel over heads). Each core computes q/k/v projections for its 4 heads,
attention, and a partial output projection; the host sums the 4 partials per
batch.

All matmul operands are bf16 (same PE rate as fp32r, half the DMA/SBUF
traffic); PSUM accumulation is fp32. Verified numerically: bf16 end-to-end
lands at ~6e-3 max rel err vs the 2e-2 gate.

Layout trick: everything is arranged so no on-device transpose is ever
needed. The host supplies hidden^T and pre-transposed weights; scores are
computed keys-major (sT = kT^T @ qT) so that the PV matmul consumes exp(sT)
directly and produces attn^T, which is exactly the lhsT layout the output
projection wants.

v2 vs the fp32r baseline: q/k/v stay entirely in SBUF between the projection
and attention phases (no DRAM roundtrip), the softmax denominator uses a
4-instruction pairwise tree on DVE (bf16 4x mode) instead of 16 adds, and
output-projection evictions run on DVE so the ACT engine only streams exp.
"""
import os
import sys

if "/opt/trn_rl_repo" not in sys.path:
    sys.path.insert(0, "/opt/trn_rl_repo")

# If a previous run crashed the NEFF execution, a fresh NRT open with this
# flag recovers the cores instead of failing with EXEC_UNIT_UNRECOVERABLE.
os.environ.setdefault("NEURON_RT_RESET_CORES", "1")

import numpy as np

B, S, D, H, HD = 2, 2048, 2048, 16, 128
NCORES = 8
GROUPS = 4            # head-groups == cores per batch
GH = H // GROUPS      # heads per core = 4
GD = GH * HD          # 512 projection cols per core
ST = 512              # s/q/o tile width
NSB = S // 128        # 16 s-blocks
NEB = D // 128        # 16 e-blocks (contraction)
NST = S // ST         # 4 s-tiles
SCALE = 1.0 / float(np.sqrt(HD))

_RUNNER = None


def _to_bf16(x: np.ndarray):
    import ml_dtypes
    return np.ascontiguousarray(x, dtype=np.float32).astype(ml_dtypes.bfloat16)


def _build_nc():
    import concourse.tile as tile
    import concourse.bass_isa as bass_isa
    from concourse import bacc, mybir

    f32 = mybir.dt.float32
    bf16 = mybir.dt.bfloat16
    Exp = mybir.ActivationFunctionType.Exp

    nc = bacc.Bacc("TRN2", target_bir_lowering=False, debug=False,
                   num_devices=NCORES)

    hT = nc.dram_tensor("hT", [D, S], bf16, kind="ExternalInput")
    wqT = nc.dram_tensor("wqT", [D, GD], bf16, kind="ExternalInput")
    wkT = nc.dram_tensor("wkT", [D, GD], bf16, kind="ExternalInput")
    wvT = nc.dram_tensor("wvT", [D, GD], bf16, kind="ExternalInput")
    woT = nc.dram_tensor("woT", [GD, D], bf16, kind="ExternalInput")
    out = nc.dram_tensor("out", [S, D], bf16, kind="ExternalOutput")

    with tile.TileContext(nc) as tc:
        with tc.tile_pool(name="persist", bufs=1) as persist:
            # q/k/v for all 4 heads stay on-chip between phases.
            qT_sb = persist.tile([128, GH, S], bf16)
            kT_sb = persist.tile([128, GH, NSB, 128], bf16)
            v_sb = persist.tile([128, NSB, GD], bf16)
            wo_sb = [persist.tile([128, D], bf16, name=f"wo{cb}")
                     for cb in range(GH)]
            woT_r = woT.rearrange("(n p) o -> p n o", p=128)

            # ---------------- phase 1: q/k/v projections ----------------
            with tc.tile_pool(name="w1", bufs=1) as w1, \
                 tc.tile_pool(name="hslab", bufs=2) as hpool, \
                 tc.tile_pool(name="ps1", bufs=8, space="PSUM") as ps1:
                wq_sb = w1.tile([128, NEB, GD], bf16)
                wk_sb = w1.tile([128, NEB, GD], bf16)
                wv_sb = w1.tile([128, NEB, GD], bf16)
                wqT_r = wqT.rearrange("(n p) d -> p n d", p=128)
                wkT_r = wkT.rearrange("(n p) d -> p n d", p=128)
                wvT_r = wvT.rearrange("(n p) d -> p n d", p=128)
                hT_r = hT.rearrange("(n p) s -> p n s", p=128)

                first_slab = hpool.tile([128, NEB, ST], bf16, tag="hslab")
                for eb in range(NEB):
                    # interleave so eb=0 pieces of the q/k path arrive first;
                    # weights lead so LDWEIGHTS can start before the rhs.
                    nc.sync.dma_start(out=wq_sb[:, eb, :],
                                      in_=wqT_r[:, eb, :])
                    nc.sync.dma_start(out=first_slab[:, eb, :],
                                      in_=hT_r[:, eb, 0:ST])
                    nc.sync.dma_start(out=wk_sb[:, eb, :],
                                      in_=wkT_r[:, eb, :])
                for eb in range(NEB):
                    nc.sync.dma_start(out=wv_sb[:, eb, :],
                                      in_=wvT_r[:, eb, :])
                for cb in range(GH):
                    nc.sync.dma_start(out=wo_sb[cb], in_=woT_r[:, cb, :])

                for st in range(NST):
                    if st == 0:
                        h_sb = first_slab
                    else:
                        h_sb = hpool.tile([128, NEB, ST], bf16, tag="hslab")
                        for eb in range(NEB):
                            nc.sync.dma_start(
                                out=h_sb[:, eb, :],
                                in_=hT_r[:, eb, st * ST:(st + 1) * ST])
                    if st == 0:
                        # eb-outer over 8 live accumulators: consume input
                        # chunks in arrival order so the PE tracks the DMA
                        # stream instead of stalling per accumulation.
                        pss = {}
                        for h in range(GH):
                            for t in range(2):
                                pss[(h, t)] = ps1.tile([128, ST], f32,
                                                       tag="ps1",
                                                       name=f"psqk{h}{t}")
                        for eb in range(NEB):
                            for h in range(GH):
                                for t, w_sb in ((0, wq_sb), (1, wk_sb)):
                                    nc.tensor.matmul(
                                        pss[(h, t)],
                                        w_sb[:, eb, h * HD:(h + 1) * HD],
                                        h_sb[:, eb, :],
                                        start=(eb == 0), stop=(eb == NEB - 1))
                        for h in range(GH):
                            for t in range(2):
                                if t == 0:
                                    tgt = qT_sb[:, h, st * ST:(st + 1) * ST]
                                else:
                                    tgt = kT_sb[:, h, st * 4:(st + 1) * 4, :]
                                nc.scalar.copy(tgt, pss[(h, t)])
                    else:
                        for h in range(GH):
                            for t, w_sb in ((0, wq_sb), (1, wk_sb)):
                                ps = ps1.tile([128, ST], f32, tag="ps1")
                                for eb in range(NEB):
                                    nc.tensor.matmul(
                                        ps, w_sb[:, eb, h * HD:(h + 1) * HD],
                                        h_sb[:, eb, :],
                                        start=(eb == 0), stop=(eb == NEB - 1))
                                if t == 0:
                                    tgt = qT_sb[:, h, st * ST:(st + 1) * ST]
                                else:
                                    tgt = kT_sb[:, h, st * 4:(st + 1) * 4, :]
                                nc.scalar.copy(tgt, ps)
                    for j in range(ST // 128):
                        ps = ps1.tile([128, GD], f32, tag="ps1")
                        for eb in range(NEB):
                            nc.tensor.matmul(
                                ps, h_sb[:, eb, j * 128:(j + 1) * 128],
                                wv_sb[:, eb, :],
                                start=(eb == 0), stop=(eb == NEB - 1))
                        nc.vector.tensor_copy(v_sb[:, st * 4 + j, :], ps)

            # ------- phase 2+3: attention fused with output projection -----
            # qt outer / heads inner: the output projection for query tile qt
            # runs as soon as all heads finished that tile, filling the PE
            # while the (ACT-bound) exp stream of the next tile runs.
            with tc.tile_pool(name="expp", bufs=2) as expp, \
                 tc.tile_pool(name="sm", bufs=2) as sm, \
                 tc.tile_pool(name="attn2", bufs=2) as attn2, \
                 tc.tile_pool(name="ev3", bufs=8) as ev3, \
                 tc.tile_pool(name="ps_s", bufs=2, space="PSUM") as ps_s, \
                 tc.tile_pool(name="acc", bufs=4, space="PSUM") as acc:

                def ph3_chunks(prev, qt_prev, j):
                    # output projection for query tile qt_prev, s-block j,
                    # one ot-column chunk per yield so the caller can weave
                    # these matmuls into the exp-latency windows of the next
                    # attention slot.
                    sb = qt_prev * (ST // 128) + j
                    for ot in range(NST):
                        po = acc.tile([128, ST], f32, tag="acc",
                                      name=f"po{sb}{ot}")
                        for cb in range(GH):
                            nc.tensor.matmul(
                                po,
                                prev[cb][:, j * 128:(j + 1) * 128],
                                wo_sb[cb][:, ot * ST:(ot + 1) * ST],
                                start=(cb == 0), stop=(cb == GH - 1))
                        ov = ev3.tile([128, ST], bf16, tag="ov",
                                      name=f"ov{sb}{ot}")
                        with nc.allow_low_precision("partials rejoin on host"):
                            nc.vector.tensor_copy(ov, po)
                        nc.scalar.dma_start(
                            out=out[sb * 128:(sb + 1) * 128,
                                    ot * ST:(ot + 1) * ST], in_=ov)
                        yield

                prev_attn = None
                for qt in range(NST):
                    attn_t = []
                    for h in range(GH):
                        qs = qT_sb[:, h, qt * ST:(qt + 1) * ST]
                        filler = (ph3_chunks(prev_attn, qt - 1, h)
                                  if prev_attn is not None else None)
                        expt = expp.tile([128, NSB, ST], bf16, tag="expt",
                                         name=f"expt{qt}{h}")
                        pa = acc.tile([128, ST], f32, tag="acc")
                        # running denominator partials, built as the exp
                        # stream lands so only ~3 adds trail the last exp
                        qsum = [None] * 4
                        run = None
                        for kb2 in range(NSB // 2):
                            ps = ps_s.tile([128, 2, ST], f32, tag="ps")
                            for half in range(2):
                                kb = kb2 * 2 + half
                                nc.tensor.matmul(ps[:, half, :],
                                                 kT_sb[:, h, kb, :], qs,
                                                 start=True, stop=True)
                            nc.scalar.activation(
                                expt[:, kb2 * 2:kb2 * 2 + 2, :], ps, Exp,
                                scale=SCALE)
                            if filler is not None and kb2 in (0, 2, 4, 7):
                                # 4 out-proj matmuls fill the exp latency
                                next(filler, None)
                            for half in range(2):
                                kb = kb2 * 2 + half
                                nc.tensor.matmul(
                                    pa, v_sb[:, kb, h * HD:(h + 1) * HD],
                                    expt[:, kb, :],
                                    start=(kb == 0), stop=(kb == NSB - 1))
                            if kb2 % 2 == 1:
                                qi = kb2 // 2
                                qsum[qi] = sm.tile([128, 2, ST], bf16,
                                                   tag="qsum",
                                                   name=f"qs{qt}{h}{qi}")
                                with nc.allow_low_precision("bf16 denom"):
                                    nc.vector.tensor_add(
                                        qsum[qi],
                                        expt[:, qi * 4:qi * 4 + 2, :],
                                        expt[:, qi * 4 + 2:qi * 4 + 4, :])
                                    if qi >= 1:
                                        prev_t = run if run is not None \
                                            else qsum[0]
                                        run = sm.tile([128, 2, ST], bf16,
                                                      tag="run",
                                                      name=f"rn{qt}{h}{qi}")
                                        nc.vector.tensor_add(run, prev_t,
                                                             qsum[qi])
                        es = sm.tile([128, ST], bf16, tag="es")
                        with nc.allow_low_precision("bf16 denom"):
                            nc.vector.tensor_add(es, run[:, 0, :],
                                                 run[:, 1, :])
                        bcsum = sm.tile([128, ST], f32, tag="bcsum")
                        nc.gpsimd.partition_all_reduce(
                            bcsum, es, 128, bass_isa.ReduceOp.add)
                        brc = sm.tile([128, ST], f32, tag="brc")
                        nc.vector.reciprocal_approx_fast(brc, bcsum)
                        at = attn2.tile([128, ST], bf16, name=f"at{h}",
                                        tag=f"at{h}")
                        with nc.allow_low_precision("bf16 attn operand"):
                            nc.vector.tensor_mul(at, pa, brc)
                        attn_t.append(at)
                        if filler is not None:
                            for _ in filler:
                                pass
                    prev_attn = attn_t
                for j in range(ST // 128):
                    for _ in ph3_chunks(prev_attn, NST - 1, j):
                        pass

    nc.compile()
    return nc


def _get_runner():
    global _RUNNER
    if _RUNNER is None:
        _RUNNER = _build_nc()
    return _RUNNER


def _prepare_in_maps(hidden_states, Wq, Wk, Wv, Wo):
    hidden = np.asarray(hidden_states, dtype=np.float32)
    hT = [_to_bf16(hidden[b].T) for b in range(B)]
    wq = np.asarray(Wq, dtype=np.float32)
    wk = np.asarray(Wk, dtype=np.float32)
    wv = np.asarray(Wv, dtype=np.float32)
    wo = np.asarray(Wo, dtype=np.float32)
    in_maps = []
    for core in range(NCORES):
        b, g = divmod(core, GROUPS)
        rows = slice(g * GD, (g + 1) * GD)
        in_maps.append({
            "hT": hT[b],
            "wqT": _to_bf16(wq[rows, :].T),
            "wkT": _to_bf16(wk[rows, :].T),
            "wvT": _to_bf16(wv[rows, :].T),
            "woT": _to_bf16(wo[:, rows].T),
        })
    return in_maps


def _run_device(in_maps, trace=False):
    from concourse.bass_utils import run_bass_kernel_spmd
    nc = _get_runner()
    try:
        return run_bass_kernel_spmd(nc, in_maps, core_ids=list(range(NCORES)),
                                    trace=trace)
    except Exception:
        # Transient device failures (rare) are recoverable by reopening the
        # backend with NEURON_RT_RESET_CORES=1. Retry once.
        try:
            import jax
            jax.clear_caches()
            try:
                jax.extend.backend.clear_backends()
            except Exception:
                jax._src.api.clear_backends()
        except Exception:
            pass
        return run_bass_kernel_spmd(nc, in_maps, core_ids=list(range(NCORES)),
                                    trace=trace)


def _numpy_reference(hidden_states, attention_mask, Wq, bq, Wk, bk, Wv, bv,
                     Wo, bo):
    """Exact fallback for inputs the fast path does not handle."""
    h = np.asarray(hidden_states, dtype=np.float32)
    mask = np.asarray(attention_mask)
    q = h @ np.asarray(Wq, np.float32).T + np.asarray(bq, np.float32)
    k = h @ np.asarray(Wk, np.float32).T + np.asarray(bk, np.float32)
    v = h @ np.asarray(Wv, np.float32).T + np.asarray(bv, np.float32)
    q = q.reshape(B, S, H, HD).transpose(0, 2, 1, 3)
    k = k.reshape(B, S, H, HD).transpose(0, 2, 1, 3)
    v = v.reshape(B, S, H, HD).transpose(0, 2, 1, 3)
    scores = (q @ k.transpose(0, 1, 3, 2)).astype(np.float32) * SCALE
    scores = np.where(mask == 0, np.float32(-1e9), scores)
    scores -= scores.max(axis=-1, keepdims=True)
    probs = np.exp(scores, dtype=np.float32)
    probs /= probs.sum(axis=-1, keepdims=True)
    attn = probs @ v
    attn = attn.transpose(0, 2, 1, 3).reshape(B, S, D)
    out = attn @ np.asarray(Wo, np.float32).T + np.asarray(bo, np.float32)
    return out.astype(np.float32)


def kernel(hidden_states, attention_mask, Wq, bq, Wk, bk, Wv, bv, Wo, bo):
    mask = np.asarray(attention_mask)
    bq_np = np.asarray(bq, dtype=np.float32)
    if (mask == 0).any() or np.any(bq_np):
        # general (never hit with the reference setup_inputs): bq shifts
        # scores per-key and a masked key changes the softmax support —
        # neither is representable in the fast path's fused layout.
        return _numpy_reference(hidden_states, attention_mask, Wq, bq, Wk,
                                bk, Wv, bv, Wo, bo)

    in_maps = _prepare_in_maps(hidden_states, Wq, Wk, Wv, Wo)
    res = _run_device(in_maps)

    # bk only adds a per-query constant to scores (softmax-invariant).
    # bv passes through the probs (rows sum to 1): out += bv @ Wo.T. bo adds.
    extra = (np.asarray(bv, np.float64) @ np.asarray(Wo, np.float64).T
             + np.asarray(bo, np.float64))
    out = np.empty((B, S, D), dtype=np.float32)
    for b in range(B):
        acc = np.zeros((S, D), dtype=np.float64)
        for g in range(GROUPS):
            acc += np.asarray(res.results[b * GROUPS + g]["out"],
                              dtype=np.float32)
        out[b] = (acc + extra).astype(np.float32)
    return out


# revision 14
# speedup vs baseline: 1.3026x; 1.3026x over previous
"""Trainium2 Bass kernel for a dense multi-head attention layer.

Problem (hardcoded shapes):
    hidden_states [2, 2048, 2048] fp32, attention_mask [2,1,1,2048] int32 (all ones),
    Wq/Wk/Wv/Wo [2048, 2048] fp32, biases [2048] fp32 (zeros in practice).
    out = MHA(hidden) with H=16 heads, head_dim=128.

Sharding: 8 cores = 2 batches x 4 head-groups (4 heads per core, tensor
parallel over heads). Each core computes q/k/v projections for its 4 heads,
attention, and a partial output projection; the host sums the 4 partials per
batch.

All matmul operands are bf16 (same PE rate as fp32r, half the DMA/SBUF
traffic); PSUM accumulation is fp32. Verified numerically: bf16 end-to-end
lands at ~6e-3 max rel err vs the 2e-2 gate.

Layout trick: everything is arranged so no on-device transpose is ever
needed. The host supplies hidden^T and pre-transposed weights; scores are
computed keys-major (sT = kT^T @ qT) so that the PV matmul consumes exp(sT)
directly and produces attn^T, which is exactly the lhsT layout the output
projection wants.

v2 vs the fp32r baseline: q/k/v stay entirely in SBUF between the projection
and attention phases (no DRAM roundtrip), the softmax denominator uses a
4-instruction pairwise tree on DVE (bf16 4x mode) instead of 16 adds, and
output-projection evictions run on DVE so the ACT engine only streams exp.
"""
import os
import sys

if "/opt/trn_rl_repo" not in sys.path:
    sys.path.insert(0, "/opt/trn_rl_repo")

# If a previous run crashed the NEFF execution, a fresh NRT open with this
# flag recovers the cores instead of failing with EXEC_UNIT_UNRECOVERABLE.
os.environ.setdefault("NEURON_RT_RESET_CORES", "1")

import numpy as np

B, S, D, H, HD = 2, 2048, 2048, 16, 128
NCORES = 8
GROUPS = 4            # head-groups == cores per batch
GH = H // GROUPS      # heads per core = 4
GD = GH * HD          # 512 projection cols per core
ST = 512              # s/q/o tile width
NSB = S // 128        # 16 s-blocks
NEB = D // 128        # 16 e-blocks (contraction)
NST = S // ST         # 4 s-tiles
SCALE = 1.0 / float(np.sqrt(HD))

_RUNNER = None


def _to_bf16(x: np.ndarray):
    import ml_dtypes
    return np.ascontiguousarray(x, dtype=np.float32).astype(ml_dtypes.bfloat16)


def _build_nc():
    import concourse.tile as tile
    import concourse.bass_isa as bass_isa
    from concourse import bacc, mybir

    f32 = mybir.dt.float32
    bf16 = mybir.dt.bfloat16
    Exp = mybir.ActivationFunctionType.Exp

    nc = bacc.Bacc("TRN2", target_bir_lowering=False, debug=False,
                   num_devices=NCORES)

    hT = nc.dram_tensor("hT", [D, S], bf16, kind="ExternalInput")
    wqT = nc.dram_tensor("wqT", [D, GD], bf16, kind="ExternalInput")
    wkT = nc.dram_tensor("wkT", [D, GD], bf16, kind="ExternalInput")
    wvT = nc.dram_tensor("wvT", [D, GD], bf16, kind="ExternalInput")
    woT = nc.dram_tensor("woT", [GD, D], bf16, kind="ExternalInput")
    out = nc.dram_tensor("out", [S, D], bf16, kind="ExternalOutput")

    with tile.TileContext(nc) as tc:
        with tc.tile_pool(name="persist", bufs=1) as persist:
            # q/k/v for all 4 heads stay on-chip between phases.
            qT_sb = persist.tile([128, GH, S], bf16)
            kT_sb = persist.tile([128, GH, NSB, 128], bf16)
            v_sb = persist.tile([128, NSB, GD], bf16)
            wo_sb = [persist.tile([128, D], bf16, name=f"wo{cb}")
                     for cb in range(GH)]
            woT_r = woT.rearrange("(n p) o -> p n o", p=128)

            # ---------------- phase 1: q/k/v projections ----------------
            with tc.tile_pool(name="w1", bufs=1) as w1, \
                 tc.tile_pool(name="hslab", bufs=2) as hpool, \
                 tc.tile_pool(name="ps1", bufs=8, space="PSUM") as ps1:
                wq_sb = w1.tile([128, NEB, GD], bf16)
                wk_sb = w1.tile([128, NEB, GD], bf16)
                wv_sb = w1.tile([128, NEB, GD], bf16)
                wqT_r = wqT.rearrange("(n p) d -> p n d", p=128)
                wkT_r = wkT.rearrange("(n p) d -> p n d", p=128)
                wvT_r = wvT.rearrange("(n p) d -> p n d", p=128)
                hT_r = hT.rearrange("(n p) s -> p n s", p=128)

                first_slab = hpool.tile([128, NEB, ST], bf16, tag="hslab")
                for eb in range(NEB):
                    # interleave so eb=0 pieces of the q/k path arrive first;
                    # weights lead so LDWEIGHTS can start before the rhs.
                    nc.sync.dma_start(out=wq_sb[:, eb, :],
                                      in_=wqT_r[:, eb, :])
                    nc.sync.dma_start(out=first_slab[:, eb, :],
                                      in_=hT_r[:, eb, 0:ST])
                    nc.sync.dma_start(out=wk_sb[:, eb, :],
                                      in_=wkT_r[:, eb, :])
                for eb in range(NEB):
                    nc.sync.dma_start(out=wv_sb[:, eb, :],
                                      in_=wvT_r[:, eb, :])
                for cb in range(GH):
                    nc.sync.dma_start(out=wo_sb[cb], in_=woT_r[:, cb, :])

                for st in range(NST):
                    if st == 0:
                        h_sb = first_slab
                    else:
                        h_sb = hpool.tile([128, NEB, ST], bf16, tag="hslab")
                        for eb in range(NEB):
                            nc.sync.dma_start(
                                out=h_sb[:, eb, :],
                                in_=hT_r[:, eb, st * ST:(st + 1) * ST])
                    if st == 0:
                        # eb-outer over 8 live accumulators: consume input
                        # chunks in arrival order so the PE tracks the DMA
                        # stream instead of stalling per accumulation.
                        pss = {}
                        for h in range(GH):
                            for t in range(2):
                                pss[(h, t)] = ps1.tile([128, ST], f32,
                                                       tag="ps1",
                                                       name=f"psqk{h}{t}")
                        for eb in range(NEB):
                            for h in range(GH):
                                for t, w_sb in ((0, wq_sb), (1, wk_sb)):
                                    nc.tensor.matmul(
                                        pss[(h, t)],
                                        w_sb[:, eb, h * HD:(h + 1) * HD],
                                        h_sb[:, eb, :],
                                        start=(eb == 0), stop=(eb == NEB - 1))
                        for h in range(GH):
                            for t in range(2):
                                if t == 0:
                                    tgt = qT_sb[:, h, st * ST:(st + 1) * ST]
                                else:
                                    tgt = kT_sb[:, h, st * 4:(st + 1) * 4, :]
                                nc.scalar.copy(tgt, pss[(h, t)])
                    else:
                        for h in range(GH):
                            for t, w_sb in ((0, wq_sb), (1, wk_sb)):
                                ps = ps1.tile([128, ST], f32, tag="ps1")
                                for eb in range(NEB):
                                    nc.tensor.matmul(
                                        ps, w_sb[:, eb, h * HD:(h + 1) * HD],
                                        h_sb[:, eb, :],
                                        start=(eb == 0), stop=(eb == NEB - 1))
                                if t == 0:
                                    tgt = qT_sb[:, h, st * ST:(st + 1) * ST]
                                else:
                                    tgt = kT_sb[:, h, st * 4:(st + 1) * 4, :]
                                nc.scalar.copy(tgt, ps)
                    for j in range(ST // 128):
                        ps = ps1.tile([128, GD], f32, tag="ps1")
                        for eb in range(NEB):
                            nc.tensor.matmul(
                                ps, h_sb[:, eb, j * 128:(j + 1) * 128],
                                wv_sb[:, eb, :],
                                start=(eb == 0), stop=(eb == NEB - 1))
                        nc.vector.tensor_copy(v_sb[:, st * 4 + j, :], ps)

            # ------- phase 2+3: attention fused with output projection -----
            # qt outer / heads inner: the output projection for query tile qt
            # runs as soon as all heads finished that tile, filling the PE
            # while the (ACT-bound) exp stream of the next tile runs.
            with tc.tile_pool(name="expp", bufs=2) as expp, \
                 tc.tile_pool(name="sm", bufs=2) as sm, \
                 tc.tile_pool(name="attn2", bufs=2) as attn2, \
                 tc.tile_pool(name="ev3", bufs=8) as ev3, \
                 tc.tile_pool(name="ps_s", bufs=2, space="PSUM") as ps_s, \
                 tc.tile_pool(name="acc", bufs=4, space="PSUM") as acc:

                def ph3_chunks(prev, qt_prev, j):
                    # output projection for query tile qt_prev, s-block j,
                    # one ot-column chunk per yield so the caller can weave
                    # these matmuls into the exp-latency windows of the next
                    # attention slot.
                    sb = qt_prev * (ST // 128) + j
                    for ot in range(NST):
                        po = acc.tile([128, ST], f32, tag="acc",
                                      name=f"po{sb}{ot}")
                        for cb in range(GH):
                            nc.tensor.matmul(
                                po,
                                prev[cb][:, j * 128:(j + 1) * 128],
                                wo_sb[cb][:, ot * ST:(ot + 1) * ST],
                                start=(cb == 0), stop=(cb == GH - 1))
                        ov = ev3.tile([128, ST], bf16, tag="ov",
                                      name=f"ov{sb}{ot}")
                        with nc.allow_low_precision("partials rejoin on host"):
                            nc.vector.tensor_copy(ov, po)
                        # sync queue: it is idle in phase 2, and a store
                        # config waiting on the eviction would block the ACT
                        # sequencer's exp stream if issued on nc.scalar.
                        nc.sync.dma_start(
                            out=out[sb * 128:(sb + 1) * 128,
                                    ot * ST:(ot + 1) * ST], in_=ov)
                        yield

                prev_attn = None
                for qt in range(NST):
                    attn_t = []
                    for h in range(GH):
                        qs = qT_sb[:, h, qt * ST:(qt + 1) * ST]
                        filler = (ph3_chunks(prev_attn, qt - 1, h)
                                  if prev_attn is not None else None)
                        expt = expp.tile([128, NSB, ST], bf16, tag="expt",
                                         name=f"expt{qt}{h}")
                        pa = acc.tile([128, ST], f32, tag="acc")
                        # running denominator partials, built as the exp
                        # stream lands so only ~3 adds trail the last exp
                        qsum = [None] * 4
                        run = None
                        for kb2 in range(NSB // 2):
                            ps = ps_s.tile([128, 2, ST], f32, tag="ps")
                            for half in range(2):
                                kb = kb2 * 2 + half
                                nc.tensor.matmul(ps[:, half, :],
                                                 kT_sb[:, h, kb, :], qs,
                                                 start=True, stop=True)
                            nc.scalar.activation(
                                expt[:, kb2 * 2:kb2 * 2 + 2, :], ps, Exp,
                                scale=SCALE)
                            if filler is not None and kb2 in (0, 2, 4, 7):
                                # 4 out-proj matmuls fill the exp latency
                                next(filler, None)
                            for half in range(2):
                                kb = kb2 * 2 + half
                                nc.tensor.matmul(
                                    pa, v_sb[:, kb, h * HD:(h + 1) * HD],
                                    expt[:, kb, :],
                                    start=(kb == 0), stop=(kb == NSB - 1))
                            if kb2 % 2 == 1:
                                qi = kb2 // 2
                                qsum[qi] = sm.tile([128, 2, ST], bf16,
                                                   tag="qsum",
                                                   name=f"qs{qt}{h}{qi}")
                                with nc.allow_low_precision("bf16 denom"):
                                    nc.vector.tensor_add(
                                        qsum[qi],
                                        expt[:, qi * 4:qi * 4 + 2, :],
                                        expt[:, qi * 4 + 2:qi * 4 + 4, :])
                                    if qi >= 1:
                                        prev_t = run if run is not None \
                                            else qsum[0]
                                        run = sm.tile([128, 2, ST], bf16,
                                                      tag="run",
                                                      name=f"rn{qt}{h}{qi}")
                                        nc.vector.tensor_add(run, prev_t,
                                                             qsum[qi])
                        es = sm.tile([128, ST], bf16, tag="es")
                        with nc.allow_low_precision("bf16 denom"):
                            nc.vector.tensor_add(es, run[:, 0, :],
                                                 run[:, 1, :])
                        bcsum = sm.tile([128, ST], f32, tag="bcsum")
                        nc.gpsimd.partition_all_reduce(
                            bcsum, es, 128, bass_isa.ReduceOp.add)
                        brc = sm.tile([128, ST], f32, tag="brc")
                        nc.vector.reciprocal_approx_fast(brc, bcsum)
                        at = attn2.tile([128, ST], bf16, name=f"at{h}",
                                        tag=f"at{h}")
                        with nc.allow_low_precision("bf16 attn operand"):
                            nc.vector.tensor_mul(at, pa, brc)
                        attn_t.append(at)
                        if filler is not None:
                            for _ in filler:
                                pass
                    prev_attn = attn_t
                for j in range(ST // 128):
                    for _ in ph3_chunks(prev_attn, NST - 1, j):
                        pass

    nc.compile()
    return nc


def _get_runner():
    global _RUNNER
    if _RUNNER is None:
        _RUNNER = _build_nc()
    return _RUNNER


def _prepare_in_maps(hidden_states, Wq, Wk, Wv, Wo):
    hidden = np.asarray(hidden_states, dtype=np.float32)
    hT = [_to_bf16(hidden[b].T) for b in range(B)]
    wq = np.asarray(Wq, dtype=np.float32)
    wk = np.asarray(Wk, dtype=np.float32)
    wv = np.asarray(Wv, dtype=np.float32)
    wo = np.asarray(Wo, dtype=np.float32)
    in_maps = []
    for core in range(NCORES):
        b, g = divmod(core, GROUPS)
        rows = slice(g * GD, (g + 1) * GD)
        in_maps.append({
            "hT": hT[b],
            "wqT": _to_bf16(wq[rows, :].T),
            "wkT": _to_bf16(wk[rows, :].T),
            "wvT": _to_bf16(wv[rows, :].T),
            "woT": _to_bf16(wo[:, rows].T),
        })
    return in_maps


def _run_device(in_maps, trace=False):
    from concourse.bass_utils import run_bass_kernel_spmd
    nc = _get_runner()
    try:
        return run_bass_kernel_spmd(nc, in_maps, core_ids=list(range(NCORES)),
                                    trace=trace)
    except Exception:
        # Transient device failures (rare) are recoverable by reopening the
        # backend with NEURON_RT_RESET_CORES=1. Retry once.
        try:
            import jax
            jax.clear_caches()
            try:
                jax.extend.backend.clear_backends()
            except Exception:
                jax._src.api.clear_backends()
        except Exception:
            pass
        return run_bass_kernel_spmd(nc, in_maps, core_ids=list(range(NCORES)),
                                    trace=trace)


def _numpy_reference(hidden_states, attention_mask, Wq, bq, Wk, bk, Wv, bv,
                     Wo, bo):
    """Exact fallback for inputs the fast path does not handle."""
    h = np.asarray(hidden_states, dtype=np.float32)
    mask = np.asarray(attention_mask)
    q = h @ np.asarray(Wq, np.float32).T + np.asarray(bq, np.float32)
    k = h @ np.asarray(Wk, np.float32).T + np.asarray(bk, np.float32)
    v = h @ np.asarray(Wv, np.float32).T + np.asarray(bv, np.float32)
    q = q.reshape(B, S, H, HD).transpose(0, 2, 1, 3)
    k = k.reshape(B, S, H, HD).transpose(0, 2, 1, 3)
    v = v.reshape(B, S, H, HD).transpose(0, 2, 1, 3)
    scores = (q @ k.transpose(0, 1, 3, 2)).astype(np.float32) * SCALE
    scores = np.where(mask == 0, np.float32(-1e9), scores)
    scores -= scores.max(axis=-1, keepdims=True)
    probs = np.exp(scores, dtype=np.float32)
    probs /= probs.sum(axis=-1, keepdims=True)
    attn = probs @ v
    attn = attn.transpose(0, 2, 1, 3).reshape(B, S, D)
    out = attn @ np.asarray(Wo, np.float32).T + np.asarray(bo, np.float32)
    return out.astype(np.float32)


def kernel(hidden_states, attention_mask, Wq, bq, Wk, bk, Wv, bv, Wo, bo):
    mask = np.asarray(attention_mask)
    bq_np = np.asarray(bq, dtype=np.float32)
    if (mask == 0).any() or np.any(bq_np):
        # general (never hit with the reference setup_inputs): bq shifts
        # scores per-key and a masked key changes the softmax support —
        # neither is representable in the fast path's fused layout.
        return _numpy_reference(hidden_states, attention_mask, Wq, bq, Wk,
                                bk, Wv, bv, Wo, bo)

    in_maps = _prepare_in_maps(hidden_states, Wq, Wk, Wv, Wo)
    res = _run_device(in_maps)

    # bk only adds a per-query constant to scores (softmax-invariant).
    # bv passes through the probs (rows sum to 1): out += bv @ Wo.T. bo adds.
    extra = (np.asarray(bv, np.float64) @ np.asarray(Wo, np.float64).T
             + np.asarray(bo, np.float64))
    out = np.empty((B, S, D), dtype=np.float32)
    for b in range(B):
        acc = np.zeros((S, D), dtype=np.float64)
        for g in range(GROUPS):
            acc += np.asarray(res.results[b * GROUPS + g]["out"],
                              dtype=np.float32)
        out[b] = (acc + extra).astype(np.float32)
    return out
